# revision 5
# baseline (speedup 1.0000x reference)
"""Trainium2 Bass kernel for GQA MultiHeadAttention (nn_MultiHeadAttention_74028056314029).

Reference computation (fp32, single device):
    Q = x @ W_q.T; K = x @ W_k.T; V = x @ W_v.T   (H=32 query heads, KV=8, G=4)
    per query head: softmax(causal(Q_h K_h^T / sqrt(D))) @ V_h
    out = hidden @ W_o.T

Sharding (8 NeuronCores, tensor-parallel over heads):
    core c owns query heads [4c, 4c+4) == KV group c (1 KV head).
    Each core computes a full-shape partial of the output projection;
    the 8 partials are summed on the host - no on-device collective.

Device scheme (bf16 matmuls, fp32 PSUM), strip-major single pass:
    - host pre-transposes x / weight shards; x chunks stream on the vector
      HWDGE ring, weights on sync, so the K/V projection starts ~2us in
    - phase A: K/V projection + Q pair-0 projection interleaved per
      contraction chunk (PSUM: 4+4 banks), then K.T/V unpack + V transpose
    - attention runs strip-major: for each 512-query strip, pair 0 then
      pair 1; scores use the zero-padded [K.T;0]/[0;K.T] K=128 trick so
      every matmul streams the full PE array; exp on ACT with 1/sqrt(D)
      fused; diagonal masked by an upper-tri multiply on GpSimd; attn@V
      appends a ones column to V so the denominator rides along (row 64)
    - the Q pair-1 projection is sliced into per-strip accumulation bursts
      that fill PE gaps during pair-0 attention; W_o for strip j fills PE
      gaps during strip j+1, so the output projection and its DMA overlap
      the attention tail instead of trailing it at half clock
    - per-strip normalize: DVE reciprocal of the PSUM den row, K=1 ones
      matmul broadcasts it across partitions, fused into the raw-hidden
      PSUM->SBUF move; no DRAM bounce
    - W_o outputs copy PSUM->SBUF alternating DVE/ACT (copy shares the
      exp ACT table) and stream out per 128-query chunk on the sync ring
"""

import os
import numpy as np
import ml_dtypes

E, H, KVH, D = 2048, 32, 8, 64
B, C = 1, 2048
G = H // KVH              # 4 query heads per core
NCORES = 8
HD_C = G * D              # 256 query head dims per core
P = 128
NE = E // P               # 16 contraction chunks
NQ = C // P               # 16 sequence chunks
SW = 512                  # strip width (one PSUM bank of fp32)
NS = C // SW              # 4 strips

BF16 = ml_dtypes.bfloat16

_CACHE: dict = {}
LAST_RESULTS = None       # BassKernelResults of the most recent run (for profiling)
TRACE = bool(int(os.environ.get("KERNEL_TRACE", "0")))


def build_bass():
    import concourse.tile as tile
    import concourse.mybir as mybir
    from concourse import bacc
    from concourse.masks import make_identity

    bf16 = mybir.dt.bfloat16
    f32 = mybir.dt.float32
    AF = mybir.ActivationFunctionType

    nc = bacc.Bacc()
    xT = nc.declare_dram_parameter("xT", [E, C], bf16, isOutput=False)
    wqT = nc.declare_dram_parameter("wqT", [E, HD_C], bf16, isOutput=False)
    wkvT = nc.declare_dram_parameter("wkvT", [E, 2 * D], bf16, isOutput=False)
    woT = nc.declare_dram_parameter("woT", [HD_C, E], bf16, isOutput=False)
    tri = nc.declare_dram_parameter("tri", [P, P], bf16, isOutput=False)
    outp = nc.declare_dram_parameter("out_part", [C, E], f32, isOutput=True)

    with tile.TileContext(nc) as tc:
        with (
            tc.tile_pool(name="big", bufs=1) as big,
            tc.tile_pool(name="expp", bufs=6) as expp,
            tc.tile_pool(name="recp", bufs=2) as recp,
            tc.tile_pool(name="outs", bufs=2) as outs,
            tc.tile_pool(name="pscp", bufs=2, space="PSUM") as pscp,
            tc.tile_pool(name="ps", bufs=2, space="PSUM") as ps,
            tc.tile_pool(name="psh", bufs=4, space="PSUM") as psh,
        ):
            # ---- persistent SBUF tensors ----
            x_sb = big.tile([P, NE, C], bf16)        # x.T: E on partitions
            wq_sb = big.tile([P, NE, HD_C], bf16)
            wkv_sb = big.tile([P, NE, 2 * D], bf16)  # [W_k | W_v] shard, transposed
            wo_sb = big.tile([P, 2, E], bf16)        # W_o shard transposed: hd on partitions
            tri_sb = big.tile([P, P], bf16)          # upper-tri ones (q>=s valid)
            ident = big.tile([P, P], bf16)
            ones_sb = big.tile([P, D], bf16)         # ones row for the K=1 PE broadcast
            kt_e = big.tile([P, C], bf16)            # [K.T ; 0] for even heads
            kt_o = big.tile([P, C], bf16)            # [0 ; K.T] for odd heads
            vt_sb = big.tile([P, C], bf16)           # V.T staged at partitions 64:128
            v_sb = big.tile([P, NQ, D + 1], bf16)    # V natural + ones column
            qt_sb = big.tile([P, 2, C], bf16)        # Q.T: head-dim on partitions
            hid_sb = big.tile([P, 2, C], bf16)       # normalized hidden.T
            ht = [big.tile([D, C], bf16, name=f"ht{m}") for m in range(2)]  # odd-head staging

            # weights on the sync ring; x chunks stream on the vector ring
            nc.sync.dma_start(out=wkv_sb, in_=wkvT[:].rearrange("(eo p) m -> p eo m", p=P))
            nc.sync.dma_start(out=wq_sb, in_=wqT[:].rearrange("(eo p) m -> p eo m", p=P))
            nc.sync.dma_start(out=tri_sb, in_=tri[:])
            nc.sync.dma_start(out=wo_sb, in_=woT[:].rearrange("(ho p) e -> p ho e", p=P))
            xTr = xT[:].rearrange("(eo p) c -> p eo c", p=P)
            for eo in range(NE):
                nc.scalar.dma_start(out=x_sb[:, eo, :], in_=xTr[:, eo, :])
            make_identity(nc, ident)
            nc.vector.memset(v_sb, 1.0)   # ones column survives; V copies overwrite the rest
            nc.vector.memset(ones_sb, 1.0)
            nc.vector.memset(kt_e[D:P, :], 0.0)
            nc.vector.memset(kt_o[0:D, :], 0.0)

            # ---- phase A: K/V projection + Q pair-0, eo-outer so x streams ----
            pkv = [pscp.tile([P, SW], f32, tag="sc", name=f"pkv{s}") for s in range(2)] + \
                  [ps.tile([P, SW], f32, tag="mm", name=f"pkv{s}") for s in (2, 3)]
            pq0 = [psh.tile([P, SW], f32, tag="hid", name=f"pq0_{s}") for s in range(NS)]
            for eo in range(NE):
                for s in range(NS):
                    nc.tensor.matmul(
                        pkv[s], wkv_sb[:, eo, :], x_sb[:, eo, s * SW:(s + 1) * SW],
                        start=(eo == 0), stop=(eo == NE - 1))
                for s in range(NS):
                    nc.tensor.matmul(
                        pq0[s], wq_sb[:, eo, 0:P], x_sb[:, eo, s * SW:(s + 1) * SW],
                        start=(eo == 0), stop=(eo == NE - 1))
            for s in range(NS):
                sl = slice(s * SW, (s + 1) * SW)
                nc.vector.tensor_copy(out=kt_e[0:D, sl], in_=pkv[s][0:D, :])
                nc.vector.tensor_copy(out=vt_sb[D:P, sl], in_=pkv[s][D:P, :])
                nc.vector.tensor_copy(out=qt_sb[:, 0, sl], in_=pq0[s])
            # odd-head copy of K.T on the other partition half (zero-padded K=128
            # keeps the score matmuls full-array so the HAM clock stays warm)
            nc.sync.dma_start(out=kt_o[D:P, :], in_=kt_e[0:D, :])
            # transpose V.T -> V natural [s, d] chunks
            for i in range(NQ):
                ptr = ps.tile([P, D], bf16, tag="mm")
                nc.tensor.transpose(ptr, vt_sb[D:P, i * P:(i + 1) * P], ident[D:P, D:P])
                nc.vector.tensor_copy(out=v_sb[:, i, 0:D], in_=ptr)

            # ---- fillers: dense PE bursts dropped into attention gaps ----
            def q1_fillers(j):
                """Q pair-1 projection for strip j as 4 accumulation bursts."""
                sl = slice(j * SW, (j + 1) * SW)
                pq1 = [None]

                def burst(k):
                    if k == 0:
                        pq1[0] = ps.tile([P, SW], f32, tag="mm", name=f"pq1_{j}")
                    for eo in range(4 * k, 4 * k + 4):
                        nc.tensor.matmul(
                            pq1[0], wq_sb[:, eo, P:2 * P], x_sb[:, eo, sl],
                            start=(eo == 0), stop=(eo == NE - 1),
                            skip_group_check=True)
                    if k == 3:
                        nc.vector.tensor_copy(out=qt_sb[:, 1, sl], in_=pq1[0])
                return [lambda k=k: burst(k) for k in range(4)]

            def wo_fillers(j):
                """Output projection for strip j, one (qc, es) unit per filler."""
                o_sb = {}

                def unit(qc, es):
                    if es == 0:
                        o_sb[qc] = outs.tile([P, E], f32, tag="o", name=f"o{qc}")
                    po = ps.tile([P, SW], f32, tag="mm", name=f"po{qc}_{es}")
                    for m in range(2):
                        nc.tensor.matmul(
                            po, hid_sb[:, m, qc * P:(qc + 1) * P],
                            wo_sb[:, m, es * SW:(es + 1) * SW],
                            start=(m == 0), stop=(m == 1), skip_group_check=True)
                    osl = o_sb[qc][:, es * SW:(es + 1) * SW]
                    if es % 2 == 0:
                        nc.vector.tensor_copy(out=osl, in_=po)
                    else:
                        nc.scalar.activation(out=osl, in_=po, func=AF.Copy)
                    if es == NS - 1:
                        nc.sync.dma_start(out=outp[qc * P:(qc + 1) * P, :], in_=o_sb[qc])
                return [lambda qc=qc, es=es: unit(qc, es)
                        for qc in range(4 * j, 4 * j + 4) for es in range(NS)]

            # ---- attention: strip-major, pair-sequential ----
            def attn_strip(pair, j, fillers):
                m = pair
                ph = {h: psh.tile([D + 1, SW], f32, tag="hid", name=f"ph{pair}_{j}_{h}")
                      for h in (0, 1)}
                fi = 0
                for i in range(4 * j + 4):
                    qlo = max(SW * j, P * i)
                    qhi = SW * (j + 1)
                    w = qhi - qlo
                    llo = qlo - SW * j
                    for h in (0, 1):      # zero-padded K=128: full PE rows
                        ktp = kt_e if h == 0 else kt_o
                        psc = pscp.tile([P, SW], f32, tag="sc")
                        nc.tensor.matmul(
                            psc[:, :w],
                            ktp[:, i * P:(i + 1) * P],
                            qt_sb[:, m, qlo:qhi],
                            start=True, stop=True)
                        et = expp.tile([P, SW], bf16, tag="exp")
                        nc.scalar.activation(out=et[:, :w], in_=psc[:, :w],
                                             func=AF.Exp, scale=0.125)
                        if qlo == P * i:   # diagonal block: zero q<s entries
                            nc.gpsimd.tensor_mul(et[:, 0:P], et[:, 0:P], tri_sb)
                        nc.tensor.matmul(
                            ph[h][:, llo:], v_sb[:, i, :], et[:, :w],
                            start=(i == 0), stop=(i == 4 * j + 3),
                            skip_group_check=True)
                    if fi < len(fillers):
                        fillers[fi]()
                        fi += 1
                while fi < len(fillers):
                    fillers[fi]()
                    fi += 1
                return ph

            def normalize(pair, j, ph):
                m = pair
                sl = slice(j * SW, (j + 1) * SW)
                for h in (0, 1):
                    rec = recp.tile([D + 1, SW], bf16, tag="rec")
                    with nc.allow_low_precision(reason="softmax denom reciprocal to bf16"):
                        nc.vector.reciprocal(out=rec[D:D + 1, :], in_=ph[h][D:D + 1, :])
                    pb = ps.tile([D, SW], f32, tag="mm", name=f"pb{pair}_{j}_{h}")
                    nc.tensor.matmul(pb, ones_sb[D:D + 1, :], rec[D:D + 1, :],
                                     start=True, stop=True, skip_group_check=True)
                    dest = hid_sb[0:D, m, sl] if h == 0 else ht[m][:, sl]
                    nc.vector.tensor_copy(out=dest, in_=ph[h][0:D, :])
                    nc.vector.tensor_mul(dest, dest, pb)
                    if h == 1:
                        nc.sync.dma_start(out=hid_sb[D:P, m, sl], in_=ht[m][:, sl])

            for j in range(NS):
                ph0 = attn_strip(0, j, q1_fillers(j))
                normalize(0, j, ph0)
                ph1 = attn_strip(1, j, wo_fillers(j - 1) if j > 0 else [])
                normalize(1, j, ph1)
            for f in wo_fillers(NS - 1):
                f()

    nc.finalize()
    return nc


def make_core_inputs(x, W_q, W_k, W_v, W_o):
    """Host-side shard + pre-transpose + bf16 cast. Returns list of in_maps."""
    x2 = np.ascontiguousarray(x.reshape(C, E).T).astype(BF16)      # [E, C]
    tri_np = np.triu(np.ones((P, P), np.float32)).astype(BF16)     # q>=s valid
    in_maps = []
    for c in range(NCORES):
        qsl = slice(c * HD_C, (c + 1) * HD_C)
        ksl = slice(c * D, (c + 1) * D)
        wq_t = np.ascontiguousarray(W_q[qsl].T).astype(BF16)                    # [E, 256]
        wkv = np.concatenate([W_k[ksl], W_v[ksl]], axis=0)                      # [128, E]
        wkv_t = np.ascontiguousarray(wkv.T).astype(BF16)                        # [E, 128]
        wo_t = np.ascontiguousarray(W_o[:, qsl].T).astype(BF16)                 # [256, E]
        in_maps.append({
            "xT": x2, "wqT": wq_t, "wkvT": wkv_t, "woT": wo_t, "tri": tri_np,
        })
    return in_maps


def kernel(x, W_q, W_k, W_v, W_o):
    global LAST_RESULTS
    from concourse.bass_utils import run_bass_kernel_spmd

    if "nc" not in _CACHE:
        _CACHE["nc"] = build_bass()
    nc = _CACHE["nc"]

    in_maps = make_core_inputs(
        np.asarray(x, np.float32), np.asarray(W_q, np.float32),
        np.asarray(W_k, np.float32), np.asarray(W_v, np.float32),
        np.asarray(W_o, np.float32))

    res = run_bass_kernel_spmd(nc, in_maps, core_ids=list(range(NCORES)), trace=TRACE)
    LAST_RESULTS = res

    out = np.zeros((C, E), np.float32)
    for r in res.results:
        out += r["out_part"]
    return out.reshape(B, C, E)


# revision 17
# speedup vs baseline: 1.1339x; 1.1339x over previous
"""Trainium2 Bass kernel for GQA MultiHeadAttention (nn_MultiHeadAttention_74028056314029).

Reference computation (fp32, single device):
    Q = x @ W_q.T; K = x @ W_k.T; V = x @ W_v.T   (H=32 query heads, KV=8, G=4)
    per query head: softmax(causal(Q_h K_h^T / sqrt(D))) @ V_h
    out = hidden @ W_o.T

Sharding (8 NeuronCores, tensor-parallel over heads):
    core c owns query heads [4c, 4c+4) == KV group c (1 KV head).
    Each core computes a full-shape partial of the output projection;
    the 8 partials are summed on the host - no on-device collective.

Device scheme (bf16 matmuls, fp32 PSUM), strip-major single pass:
    - x chunks stream with early chunks split across several DMA engines so
      the K/V projection starts ~4us in; K/V + Q pair-0 projections are
      interleaved per contraction chunk (PSUM 4+4 banks)
    - attention runs strip-major: for each 512-query strip, pair 0 then
      pair 1; scores use the zero-padded [K.T;0]/[0;K.T] K=128 trick so
      every matmul streams the full PE array; exp on ACT with 1/sqrt(D)
      fused; diagonal masked by an upper-tri multiply on GpSimd; attn@V
      appends a ones column to V so the denominator rides along (row 64)
    - the Q pair-1 projection is sliced into per-strip accumulation bursts
      that fill PE gaps during pair-0 attention; W_o for strip j fills PE
      gaps during strip j+1 so the output projection and its DMA overlap
      attention instead of trailing at half clock
    - per-(pair,strip) normalize: the 1-lane den rows bounce through DRAM
      into [64,16] so the DVE reciprocal is free-size-cheap; the K=1 ones
      matmul broadcast + hidden multiply run later as PE-gap fillers
    - W_o outputs copy PSUM->SBUF alternating DVE/ACT (copy shares the
      exp ACT table) and stream out per 128-query chunk on the sync ring
"""

import os
import numpy as np
import ml_dtypes

E, H, KVH, D = 2048, 32, 8, 64
B, C = 1, 2048
G = H // KVH              # 4 query heads per core
NCORES = 8
HD_C = G * D              # 256 query head dims per core
P = 128
NE = E // P               # 16 contraction chunks
NQ = C // P               # 16 sequence chunks
SW = 512                  # strip width (one PSUM bank of fp32)
NS = C // SW              # 4 strips

BF16 = ml_dtypes.bfloat16

_CACHE: dict = {}
LAST_RESULTS = None       # BassKernelResults of the most recent run (for profiling)
TRACE = bool(int(os.environ.get("KERNEL_TRACE", "0")))


def build_bass():
    import concourse.tile as tile
    import concourse.mybir as mybir
    from concourse import bacc
    from concourse.masks import make_identity

    bf16 = mybir.dt.bfloat16
    f32 = mybir.dt.float32
    AF = mybir.ActivationFunctionType

    nc = bacc.Bacc()
    xT = nc.declare_dram_parameter("xT", [E, C], bf16, isOutput=False)
    wqT = nc.declare_dram_parameter("wqT", [E, HD_C], bf16, isOutput=False)
    wkvT = nc.declare_dram_parameter("wkvT", [E, 2 * D], bf16, isOutput=False)
    woT = nc.declare_dram_parameter("woT", [HD_C, E], bf16, isOutput=False)
    tri = nc.declare_dram_parameter("tri", [P, P], bf16, isOutput=False)
    outp = nc.declare_dram_parameter("out_part", [C, E], f32, isOutput=True)
    scr_den = nc.dram_tensor("scr_den", [NS, 2, 2, SW], bf16)
    scr_rec = nc.dram_tensor("scr_rec", [NS, 2, 2, SW], bf16)

    with tile.TileContext(nc) as tc:
        with (
            tc.tile_pool(name="big", bufs=1) as big,
            tc.tile_pool(name="expp", bufs=6) as expp,
            tc.tile_pool(name="denp", bufs=2) as denp,
            tc.tile_pool(name="ddp", bufs=2) as ddp,
            tc.tile_pool(name="recp", bufs=4) as recp,
            tc.tile_pool(name="outs", bufs=2) as outs,
            tc.tile_pool(name="pscp", bufs=2, space="PSUM") as pscp,
            tc.tile_pool(name="ps", bufs=2, space="PSUM") as ps,
            tc.tile_pool(name="psh", bufs=4, space="PSUM") as psh,
        ):
            # ---- persistent SBUF tensors ----
            x_sb = big.tile([P, NE, C], bf16)        # x.T: E on partitions
            wq_sb = big.tile([P, NE, HD_C], bf16)
            wkv_sb = big.tile([P, NE, 2 * D], bf16)  # [W_k | W_v] shard, transposed
            wo_sb = big.tile([P, 2, E], bf16)        # W_o shard transposed: hd on partitions
            tri_sb = big.tile([P, P], bf16)          # upper-tri ones (q>=s valid)
            ident = big.tile([P, P], bf16)
            ones_sb = big.tile([P, D], bf16)         # ones row for the K=1 PE broadcast
            kt_e = big.tile([P, C], bf16)            # [K.T ; 0] for even heads
            kt_o = big.tile([P, C], bf16)            # [0 ; K.T] for odd heads
            vt_sb = big.tile([P, C], bf16)           # V.T staged at partitions 64:128
            v_sb = big.tile([P, NQ, D + 1], bf16)    # V natural + ones column
            qt_sb = big.tile([P, 2, C], bf16)        # Q.T: head-dim on partitions
            hid_sb = big.tile([P, 2, C], bf16)       # normalized hidden.T
            ht = [big.tile([D, C], bf16, name=f"ht{m}") for m in range(2)]

            # weights split x8 so the first K/V matmul starts ~4us in;
            # x chunks split so early chunks land just behind them
            wkvr = wkvT[:].rearrange("(eo p) m -> p eo m", p=P)
            wqr = wqT[:].rearrange("(eo p) m -> p eo m", p=P)
            nc.sync.dma_start(out=tri_sb, in_=tri[:])
            for h8 in range(8):
                ring = nc.sync if h8 % 2 == 0 else nc.scalar
                ring.dma_start(out=wkv_sb[:, 2 * h8:2 * (h8 + 1), :],
                               in_=wkvr[:, 2 * h8:2 * (h8 + 1), :])
            for h8 in range(8):
                ring = nc.scalar if h8 % 2 == 0 else nc.sync
                ring.dma_start(out=wq_sb[:, 2 * h8:2 * (h8 + 1), :],
                               in_=wqr[:, 2 * h8:2 * (h8 + 1), :])
            xTr = xT[:].rearrange("(eo p) c -> p eo c", p=P)
            splits = [4] * 4 + [2] * 4 + [1] * 8
            for eo in range(NE):
                n = splits[eo]
                w = C // n
                for k in range(n):
                    ring = nc.scalar if (eo + k) % 2 == 0 else nc.sync
                    ring.dma_start(out=x_sb[:, eo, k * w:(k + 1) * w],
                                   in_=xTr[:, eo, k * w:(k + 1) * w])
            wor = woT[:].rearrange("(ho p) e -> p ho e", p=P)
            for h4 in range(4):
                ring = nc.scalar if h4 % 2 == 0 else nc.sync
                ring.dma_start(out=wo_sb[:, h4 // 2, (h4 % 2) * 1024:(h4 % 2 + 1) * 1024],
                               in_=wor[:, h4 // 2, (h4 % 2) * 1024:(h4 % 2 + 1) * 1024])
            make_identity(nc, ident)
            nc.vector.memset(v_sb, 1.0)   # ones column survives; V copies overwrite the rest
            nc.vector.memset(ones_sb, 1.0)
            nc.vector.memset(kt_e[D:P, :], 0.0)
            nc.vector.memset(kt_o[0:D, :], 0.0)

            # ---- phase A: K/V projection + Q pair-0, eo-outer so x streams ----
            pkv = [pscp.tile([P, SW], f32, tag="sc", name=f"pkv{s}") for s in range(2)] + \
                  [ps.tile([P, SW], f32, tag="mm", name=f"pkv{s}") for s in (2, 3)]
            pq0 = [psh.tile([P, SW], f32, tag="hid", name=f"pq0_{s}") for s in range(NS)]
            for eo in range(NE):
                for s in range(NS):
                    nc.tensor.matmul(
                        pkv[s], wkv_sb[:, eo, :], x_sb[:, eo, s * SW:(s + 1) * SW],
                        start=(eo == 0), stop=(eo == NE - 1))
                for s in range(NS):
                    nc.tensor.matmul(
                        pq0[s], wq_sb[:, eo, 0:P], x_sb[:, eo, s * SW:(s + 1) * SW],
                        start=(eo == 0), stop=(eo == NE - 1))
            for s in range(NS):
                sl = slice(s * SW, (s + 1) * SW)
                nc.vector.tensor_copy(out=kt_e[0:D, sl], in_=pkv[s][0:D, :])
                nc.vector.tensor_copy(out=vt_sb[D:P, sl], in_=pkv[s][D:P, :])
                nc.vector.tensor_copy(out=qt_sb[:, 0, sl], in_=pq0[s])
            # odd-head copy of K.T on the other partition half (zero-padded K=128
            # keeps the score matmuls full-array so the HAM clock stays warm)
            nc.sync.dma_start(out=kt_o[D:P, :], in_=kt_e[0:D, :])
            # transpose V.T -> V natural [s, d] chunks
            for i in range(NQ):
                ptr = ps.tile([P, D], bf16, tag="mm")
                nc.tensor.transpose(ptr, vt_sb[D:P, i * P:(i + 1) * P], ident[D:P, D:P])
                nc.vector.tensor_copy(out=v_sb[:, i, 0:D], in_=ptr)

            # ---- fillers: dense PE bursts dropped into attention gaps ----
            def q1_fillers(j):
                """Q pair-1 projection for strip j as 4 accumulation bursts."""
                sl = slice(j * SW, (j + 1) * SW)
                pq1 = [None]

                def burst(k):
                    if k == 0:
                        pq1[0] = ps.tile([P, SW], f32, tag="mm", name=f"pq1_{j}")
                    for eo in range(4 * k, 4 * k + 4):
                        nc.tensor.matmul(
                            pq1[0], wq_sb[:, eo, P:2 * P], x_sb[:, eo, sl],
                            start=(eo == 0), stop=(eo == NE - 1),
                            skip_group_check=True)
                    if k == 3:
                        nc.vector.tensor_copy(out=qt_sb[:, 1, sl], in_=pq1[0])
                return [lambda k=k: burst(k) for k in range(4)]

            # per-(pair,strip) 1/den: the 1-lane den rows bounce through DRAM
            # into [64,16] so the exact DVE reciprocal is free-size-cheap
            rec_tiles = {}

            def norm_chain(pair, j, ph):
                for h in (0, 1):
                    den = denp.tile([D + 1, SW], bf16, tag="den",
                                    name=f"den{pair}_{j}_{h}")
                    nc.vector.tensor_copy(out=den[D:D + 1, :], in_=ph[h][D:D + 1, :])
                    nc.sync.dma_start(out=scr_den[j, pair, h], in_=den[D:D + 1, :])
                dd = ddp.tile([D, NQ], bf16, tag="dd", name=f"dd{pair}_{j}")
                nc.gpsimd.dma_start(
                    out=dd, in_=scr_den[j, pair].rearrange("h (p o) -> (h p) o", p=32))
                rr = ddp.tile([D, NQ], bf16, tag="rr", name=f"rr{pair}_{j}")
                with nc.allow_low_precision(reason="softmax denom reciprocal"):
                    nc.vector.reciprocal(out=rr, in_=dd)
                nc.gpsimd.dma_start(
                    out=scr_rec[j, pair].rearrange("h (p o) -> (h p) o", p=32), in_=rr)
                for h in (0, 1):
                    rec = recp.tile([D + 1, SW], bf16, tag="rec",
                                    name=f"rec{pair}_{j}_{h}")
                    rec_tiles[(pair, j, h)] = rec
                    nc.sync.dma_start(out=rec[D:D + 1, :], in_=scr_rec[j, pair, h])

            def norm_units(pair, j, ph):
                """Broadcast 1/den and normalize raw hidden; one unit per head."""
                m = pair
                sl = slice(j * SW, (j + 1) * SW)

                def unit(h):
                    rec = rec_tiles[(pair, j, h)]
                    pb = ps.tile([D, SW], f32, tag="mm", name=f"pb{pair}_{j}_{h}")
                    nc.tensor.matmul(pb, ones_sb[D:D + 1, :], rec[D:D + 1, :],
                                     start=True, stop=True, skip_group_check=True)
                    dest = hid_sb[0:D, m, sl] if h == 0 else ht[m][:, sl]
                    nc.vector.tensor_copy(out=dest, in_=ph[h][0:D, :])
                    nc.vector.tensor_mul(dest, dest, pb)
                    if h == 1:
                        nc.sync.dma_start(out=hid_sb[D:P, m, sl], in_=ht[m][:, sl])
                return [lambda h=h: unit(h) for h in (0, 1)]

            def wo_fillers(j):
                """Output projection for strip j, one (qc, es) unit per filler."""
                o_sb = {}

                def unit(qc, es):
                    if es == 0:
                        o_sb[qc] = outs.tile([P, E], f32, tag="o", name=f"o{qc}")
                    po = ps.tile([P, SW], f32, tag="mm", name=f"po{qc}_{es}")
                    for m in range(2):
                        nc.tensor.matmul(
                            po, hid_sb[:, m, qc * P:(qc + 1) * P],
                            wo_sb[:, m, es * SW:(es + 1) * SW],
                            start=(m == 0), stop=(m == 1), skip_group_check=True)
                    osl = o_sb[qc][:, es * SW:(es + 1) * SW]
                    if es % 2 == 0:
                        nc.vector.tensor_copy(out=osl, in_=po)
                    else:
                        nc.scalar.activation(out=osl, in_=po, func=AF.Copy)
                    if es == NS - 1:
                        nc.sync.dma_start(out=outp[qc * P:(qc + 1) * P, :], in_=o_sb[qc])
                return [lambda qc=qc, es=es: unit(qc, es)
                        for qc in range(4 * j, 4 * j + 4) for es in range(NS)]

            # ---- attention: strip-major, pair-sequential ----
            def attn_strip(pair, j, fillers):
                m = pair
                ph = {h: psh.tile([D + 1, SW], f32, tag="hid", name=f"ph{pair}_{j}_{h}")
                      for h in (0, 1)}
                fi = 0
                for i in range(4 * j + 4):
                    qlo = max(SW * j, P * i)
                    qhi = SW * (j + 1)
                    w = qhi - qlo
                    llo = qlo - SW * j
                    for h in (0, 1):      # zero-padded K=128: full PE rows
                        ktp = kt_e if h == 0 else kt_o
                        psc = pscp.tile([P, SW], f32, tag="sc")
                        nc.tensor.matmul(
                            psc[:, :w],
                            ktp[:, i * P:(i + 1) * P],
                            qt_sb[:, m, qlo:qhi],
                            start=True, stop=True)
                        et = expp.tile([P, SW], bf16, tag="exp")
                        nc.scalar.activation(out=et[:, :w], in_=psc[:, :w],
                                             func=AF.Exp, scale=0.125)
                        if qlo == P * i:   # diagonal block: zero q<s entries
                            nc.gpsimd.tensor_mul(et[:, 0:P], et[:, 0:P], tri_sb)
                        nc.tensor.matmul(
                            ph[h][:, llo:], v_sb[:, i, :], et[:, :w],
                            start=(i == 0), stop=(i == 4 * j + 3),
                            skip_group_check=True)
                    if fi < len(fillers):
                        fillers[fi]()
                        fi += 1
                while fi < len(fillers):
                    fillers[fi]()
                    fi += 1
                return ph

            phs = {}
            for j in range(NS):
                # pair 0: Q pair-1 bursts, then normalize of strip j-1 pair 1
                # (bursts first give the den->1/den DRAM bounce time to land)
                f0 = q1_fillers(j)
                if j > 0:
                    f0 += norm_units(1, j - 1, phs[(1, j - 1)])
                phs[(0, j)] = attn_strip(0, j, f0)
                norm_chain(0, j, phs[(0, j)])
                # pair 1: W_o of strip j-1, then normalize of strip j pair 0
                f1 = wo_fillers(j - 1) if j > 0 else []
                f1 += norm_units(0, j, phs[(0, j)])
                phs[(1, j)] = attn_strip(1, j, f1)
                norm_chain(1, j, phs[(1, j)])
            for f in norm_units(1, NS - 1, phs[(1, NS - 1)]):
                f()
            for f in wo_fillers(NS - 1):
                f()

    nc.finalize()
    return nc


def make_core_inputs(x, W_q, W_k, W_v, W_o):
    """Host-side shard + pre-transpose + bf16 cast. Returns list of in_maps."""
    x2 = np.ascontiguousarray(x.reshape(C, E).T).astype(BF16)      # [E, C]
    tri_np = np.triu(np.ones((P, P), np.float32)).astype(BF16)     # q>=s valid
    in_maps = []
    for c in range(NCORES):
        qsl = slice(c * HD_C, (c + 1) * HD_C)
        ksl = slice(c * D, (c + 1) * D)
        wq_t = np.ascontiguousarray(W_q[qsl].T).astype(BF16)                    # [E, 256]
        wkv = np.concatenate([W_k[ksl], W_v[ksl]], axis=0)                      # [128, E]
        wkv_t = np.ascontiguousarray(wkv.T).astype(BF16)                        # [E, 128]
        wo_t = np.ascontiguousarray(W_o[:, qsl].T).astype(BF16)                 # [256, E]
        in_maps.append({
            "xT": x2, "wqT": wq_t, "wkvT": wkv_t, "woT": wo_t, "tri": tri_np,
        })
    return in_maps


def kernel(x, W_q, W_k, W_v, W_o):
    global LAST_RESULTS
    from concourse.bass_utils import run_bass_kernel_spmd

    if "nc" not in _CACHE:
        _CACHE["nc"] = build_bass()
    nc = _CACHE["nc"]

    in_maps = make_core_inputs(
        np.asarray(x, np.float32), np.asarray(W_q, np.float32),
        np.asarray(W_k, np.float32), np.asarray(W_v, np.float32),
        np.asarray(W_o, np.float32))

    res = run_bass_kernel_spmd(nc, in_maps, core_ids=list(range(NCORES)), trace=TRACE)
    LAST_RESULTS = res

    out = np.zeros((C, E), np.float32)
    for r in res.results:
        out += r["out_part"]
    return out.reshape(B, C, E)
